# revision 40
# baseline (speedup 1.0000x reference)
"""NetVLAD Trainium2 Bass kernel.

Full inputs in, full output out. Data-parallel over batch N=64 across 8
NeuronCores (8 samples per core); conv weight and centroids replicated.

Per-sample algorithm (mathematically equal to the reference, never
materializing the channel-normalized x):
  X = x[n]  [D=128, P=4800]  (D on SBUF partitions, contiguous in HBM)
  For each 128-wide chunk of P (p on partitions after a PE transpose):
    ss[p]    = sum_d X[d,p]^2
    inv_s    = ss^-0.5                   (DVE pow — keeps the ACT table
                                          set fixed: only Copy/Square/Exp)
    logitsT  = X_c^T @ Wt                (PE)
    e        = exp(logitsT * inv_s)      (softmax max-subtraction skipped:
                                          |logits*inv_s| <= ~1.2)
    sb       = e * (inv_s / Z),  Z = sum_k e
    acc     += [sb | e]^T @ [X_c^T | 1/Z]   (PE, PSUM accumulate)
  agg      = acc[0:64, 0:128];  sum_sa = acc[64:128, 128]
  vlad     = agg - sum_sa * centroids, then intra + global L2 norm.

Pipelining: per-chunk scalar work is batched into whole-sample ops
(one Square, one reduce, one Exp, ...); the accumulate matmuls of
sample n-2 are emitted between pass A of sample n so the PE never
waits on the scalar chain. The [sb|e] and [XT|1/Z] operands are bf16
(FWL fast weight load; f32 PSUM accumulation).
"""

import sys

if "/opt/trn_rl_repo" not in sys.path:
    sys.path.insert(0, "/opt/trn_rl_repo")

import numpy as np
from contextlib import ExitStack

N, D, HW, K = 64, 128, 4800, 64
NCORES = 8
NS = N // NCORES  # samples per core

CHUNKS = [(i * 128, min(128, HW - i * 128)) for i in range((HW + 127) // 128)]
NCH = len(CHUNKS)  # 38: 37 full + one 64-wide

_CACHE = {}


def _patch_act_tables():
    """Steer bacc's ACT table-set placement to the one set that covers
    every function we use (ln/exp/square/copy) so the kernel pays a single
    ACT_TABLE_LOAD instead of thrashing between per-anchor sets."""
    if _CACHE.get("act_patched"):
        return
    from concourse import bacc, mybir

    orig = bacc.get_activation_tables
    AF = mybir.ActivationFunctionType
    combo = "natural_log_exp_and_others"

    def patched(arch):
        t = {k: set(v) for k, v in orig(arch).items()}
        if combo in t:
            for name in t:
                if name != combo:
                    t[name] = t[name] - {AF.Ln, AF.Exp}
        return t

    bacc.get_activation_tables = patched
    _CACHE["act_patched"] = True


def _build_nc():
    import concourse.tile as tile
    from concourse import bacc, mybir

    _patch_act_tables()

    nc = bacc.Bacc(
        "TRN2",
        target_bir_lowering=False,
        debug=False,
        enable_asserts=False,
        num_devices=NCORES,
    )
    x_ap = nc.dram_tensor("x", [NS, D, HW], mybir.dt.float32, kind="ExternalInput").ap()
    wt_ap = nc.dram_tensor("wt", [D, K], mybir.dt.float32, kind="ExternalInput").ap()
    cent_ap = nc.dram_tensor(
        "cent", [K, D], mybir.dt.float32, kind="ExternalInput"
    ).ap()
    out_ap = nc.dram_tensor(
        "out", [NS, K, D], mybir.dt.float32, kind="ExternalOutput"
    ).ap()

    with tile.TileContext(nc) as tc:
        with ExitStack() as ctx:
            _body(ctx, tc, out_ap, x_ap, wt_ap, cent_ap)
    nc.compile()
    return nc


def _body(ctx, tc, out_ap, x_ap, wt_ap, cent_ap):
    import concourse.bass as bass
    from concourse import masks, mybir

    nc = tc.nc
    f32 = mybir.dt.float32
    bf16 = mybir.dt.bfloat16
    AF = mybir.ActivationFunctionType
    ALU = mybir.AluOpType
    X_AX = mybir.AxisListType.X

    singles = ctx.enter_context(tc.tile_pool(name="singles", bufs=1))
    xpool = ctx.enter_context(tc.tile_pool(name="xpool", bufs=2))
    xtrpool = ctx.enter_context(tc.tile_pool(name="xtrpool", bufs=4))
    ebpool = ctx.enter_context(tc.tile_pool(name="ebpool", bufs=2))
    sbtpool = ctx.enter_context(tc.tile_pool(name="sbtpool", bufs=4))
    lpool = ctx.enter_context(tc.tile_pool(name="lpool", bufs=2))
    scrpool = ctx.enter_context(tc.tile_pool(name="scrpool", bufs=2))
    smalls = ctx.enter_context(tc.tile_pool(name="smalls", bufs=3))
    tails = ctx.enter_context(tc.tile_pool(name="tails", bufs=1))
    pp_xt = ctx.enter_context(tc.tile_pool(name="pp_xt", bufs=6, space="PSUM"))
    pp_acc = ctx.enter_context(tc.tile_pool(name="pp_acc", bufs=1, space="PSUM"))
    pp_tiny = ctx.enter_context(tc.tile_pool(name="pp_tiny", bufs=1, space="PSUM"))

    def bcast(ap, n):
        # append a step-0 free dim: [..., n] broadcast view
        return bass.AP(tensor=ap.tensor, offset=ap.offset, ap=list(ap.ap) + [[0, n]])

    def mid_bcast(ap, n):
        # [p, f] -> [p, n, f] with step-0 middle dim
        return bass.AP(
            tensor=ap.tensor,
            offset=ap.offset,
            ap=[ap.ap[0], [0, n]] + list(ap.ap[1:]),
        )

    # constants
    ident = singles.tile([128, 128], f32)
    masks.make_identity(nc, ident[:])
    # fused rhs for pass A: [identity | Wt] — one matmul yields [X_c^T | logits]
    # (bf16 so the stationary X loads get FWL)
    wt_f32 = singles.tile([D, K], f32)
    nc.sync.dma_start(out=wt_f32[:], in_=wt_ap[:])
    identwt = singles.tile([128, 192], bf16)
    masks.make_identity(nc, identwt[:, 0:128])
    nc.vector.tensor_copy(identwt[:, 128:192], wt_f32[:])
    cent_s = singles.tile([K, D], f32)
    nc.sync.dma_start(out=cent_s[:], in_=cent_ap[:])
    ones_col = singles.tile([K, 1], f32)
    nc.vector.memset(ones_col[:], 1.0)
    ones_row = singles.tile([1, K], f32)
    nc.vector.memset(ones_row[:], 1.0)

    GRP = 2  # fused-matmul chunks per PSUM bank (finer -> more PE runway)
    groups = []
    c0 = 0
    while c0 < NCH:
        groups.append(list(range(c0, min(c0 + GRP, NCH))))
        c0 += GRP

    state = {}  # per-sample live tiles

    def emit_load_and_passA(n):
        xs = xpool.tile([D, HW], f32, tag="xs")
        nc.sync.dma_start(out=xs[:, 0 : HW // 2], in_=x_ap[n, :, 0 : HW // 2])
        nc.sync.dma_start(out=xs[:, HW // 2 :], in_=x_ap[n, :, HW // 2 :])
        # bf16 X feeds the fused matmul (FWL fast weight load)
        xbf = xpool.tile([D, HW], bf16, tag="xbf")
        nc.gpsimd.tensor_copy(xbf[:, 0 : HW // 2], xs[:, 0 : HW // 2])
        nc.gpsimd.tensor_copy(xbf[:, HW // 2 :], xs[:, HW // 2 :])

        # [XT | s] per chunk (bf16): cols 0:128 = X_c^T, col 128 = ||x_p||
        xtr = xtrpool.tile([128, NCH, 129], bf16, tag="xtr")
        # softmax numerators e (bf16, contiguous for the 2x reduce)
        et = ebpool.tile([128, NCH, K], bf16, tag="et")
        # sb = e * inv_s/Z — the acc matmul's stationary operand
        sbt = sbtpool.tile([128, NCH, K], bf16, tag="sbt")
        # raw logits stash (bf16)
        lgs = lpool.tile([128, NCH, K], bf16, tag="lgs")
        # XT^2 scratch (bf16 — ss reduce gets the 2x DVE mode)
        x2t = scrpool.tile([128, NCH * 128], bf16, tag="x2t")
        # scaled-logits scratch (bf16)
        slgt = scrpool.tile([128, NCH * K], bf16, tag="slgt")

        for gi, grp in enumerate(groups):
            gn = len(grp)
            # one fused matmul per chunk: out cols 0:128 = X_c^T, 128:192 =
            # logits. 256-col stride keeps each 192-col output in one bank.
            xt_p = pp_xt.tile([128, GRP, 256], f32, tag="xt")
            for j, c in enumerate(grp):
                p0, w = CHUNKS[c]
                x_c = xbf[:, p0 : p0 + w]
                nc.tensor.matmul(
                    xt_p[:w, j, 0:192],
                    lhsT=x_c,
                    rhs=identwt[:],
                    start=True,
                    stop=True,
                )
            gc = grp[0]
            # alternate evacuation between DVE and ACT so neither engine's
            # batch work starves the PE's PSUM recycling
            if gi % 2 == 0:
                nc.vector.tensor_copy(
                    xtr[:, gc : gc + gn, 0:128], xt_p[:, 0:gn, 0:128]
                )
                nc.scalar.copy(lgs[:, gc : gc + gn, :], xt_p[:, 0:gn, 128:192])
            else:
                nc.scalar.copy(xtr[:, gc : gc + gn, 0:128], xt_p[:, 0:gn, 0:128])
                nc.vector.tensor_copy(lgs[:, gc : gc + gn, :], xt_p[:, 0:gn, 128:192])

        state[n] = (xs, xtr, et, sbt, lgs, x2t, slgt)

    def emit_scalars(n):
        xs, xtr, et, sbt, lgs, x2t, slgt = state[n]
        ss = smalls.tile([128, NCH], f32, tag="ss")
        zz = smalls.tile([128, NCH], f32, tag="zz")
        is_ = smalls.tile([128, NCH], f32, tag="is")
        tsc = smalls.tile([128, NCH], bf16, tag="tsc")

        # everything split per half-sample: shorter dependency links, and
        # the static per-engine schedule interleaves across samples better
        halves = [(0, NCH // 2), (NCH // 2, NCH)]
        lns = smalls.tile([128, NCH], f32, tag="lns")
        x2vf = x2t[:].rearrange("p (c d) -> p c d", c=NCH)
        for h0, h1 in halves:
            nc.scalar.activation(x2vf[:, h0:h1, :], xtr[:, h0:h1, 0:128], AF.Square)
            nc.vector.tensor_reduce(
                out=ss[:, h0:h1], in_=x2vf[:, h0:h1, :], axis=X_AX, op=ALU.add
            )
            # inv_s = exp(-0.5*ln(ss)); Ln+Exp live in one ACT table set
            nc.scalar.activation(lns[:, h0:h1], ss[:, h0:h1], AF.Ln)
            nc.scalar.activation(is_[:, h0:h1], lns[:, h0:h1], AF.Exp, scale=-0.5)
            # s = ss * inv_s = ||x_p||, into col 128 of each xtr chunk (the
            # acc matmul's rhs column turning sb into sum_sa)
            nc.gpsimd.tensor_tensor(
                out=xtr[:, h0:h1, 128],
                in0=ss[:, h0:h1],
                in1=is_[:, h0:h1],
                op=ALU.mult,
            )

        for h0, h1 in halves:
            slg = slgt[:, h0 * K : h1 * K].rearrange("p (c k) -> p c k", c=h1 - h0)
            nc.gpsimd.tensor_tensor(
                out=slg,
                in0=lgs[:, h0:h1, :],
                in1=bcast(is_[:, h0:h1], K),
                op=ALU.mult,
            )
            nc.scalar.activation(et[:, h0:h1, :], slg, AF.Exp)
            nc.vector.tensor_reduce(
                out=zz[:, h0:h1], in_=et[:, h0:h1, :], axis=X_AX, op=ALU.add
            )
            rr = smalls.tile([128, NCH // 2], f32, tag="rr")
            nc.vector.reciprocal(rr[:], zz[:, h0:h1])
            # t = inv_s / Z
            nc.gpsimd.tensor_tensor(
                out=tsc[:, h0:h1], in0=is_[:, h0:h1], in1=rr[:], op=ALU.mult
            )
            # sb = e * t
            nc.gpsimd.tensor_tensor(
                out=sbt[:, h0:h1, :],
                in0=et[:, h0:h1, :],
                in1=bcast(tsc[:, h0:h1], K),
                op=ALU.mult,
            )

    def emit_passC(n, agg_all, ssa_all):
        xs, xtr, et, sbt, lgs, x2t, slgt = state.pop(n)
        acc_p = pp_acc.tile([K, 129], f32, tag="acc")
        for c, (p0, w) in enumerate(CHUNKS):
            nc.tensor.matmul(
                acc_p[:, :],
                lhsT=sbt[:w, c, :],
                rhs=xtr[:w, c, :],
                start=(c == 0),
                stop=(c == NCH - 1),
            )
        # evacuate: agg = cols 0:128; sum_sa = col 128
        nc.vector.tensor_copy(agg_all[:, n, :], acc_p[:, 0:D])
        nc.scalar.copy(ssa_all[:, n : n + 1], acc_p[:, 128:129])

    # batched across all samples
    agg_all = tails.tile([K, NS, D], f32)
    ssa_all = tails.tile([K, NS], f32)

    def emit_tail(n0, n1):
        nn = n1 - n0
        agg_h = agg_all[:, n0:n1, :]
        ssa_h = ssa_all[:, n0:n1]
        vl = tails.tile([K, nn, D], f32, tag=f"t_vl{n0}")
        vsq = tails.tile([K, nn * D], f32, tag=f"t_vsq{n0}")
        q = tails.tile([K, nn], f32, tag=f"t_q{n0}")
        qm = tails.tile([K, nn], f32, tag=f"t_qm{n0}")
        isq = tails.tile([K, nn], f32, tag=f"t_isq{n0}")
        isq2 = tails.tile([K, nn], f32, tag=f"t_isq2{n0}")
        u = tails.tile([K, nn], f32, tag=f"t_u{n0}")
        gisr = tails.tile([1, nn], f32, tag=f"t_gisr{n0}")
        gb = tails.tile([K, nn], f32, tag=f"t_gb{n0}")
        sall = tails.tile([K, nn], f32, tag=f"t_s{n0}")
        vf = tails.tile([K, nn, D], f32, tag=f"t_vf{n0}")

        # vl = agg - ssa * cent
        nc.gpsimd.tensor_tensor(
            out=vl[:], in0=bcast(ssa_h, D), in1=mid_bcast(cent_s[:], nn), op=ALU.mult
        )
        nc.vector.tensor_tensor(out=vl[:], in0=agg_h, in1=vl[:], op=ALU.subtract)
        # q = rowsum(vl^2) per (k, n)
        vsqv = vsq[:].rearrange("k (n d) -> k n d", n=nn)
        nc.scalar.activation(vsqv, vl[:], AF.Square)
        nc.vector.tensor_reduce(out=q[:], in_=vsqv, axis=X_AX, op=ALU.add)
        nc.vector.tensor_scalar_max(qm[:], q[:], 1e-24)
        lq = tails.tile([K, nn], f32, tag=f"t_lq{n0}")
        nc.scalar.activation(lq[:], qm[:], AF.Ln)
        nc.scalar.activation(isq[:], lq[:], AF.Exp, scale=-0.5)
        # g = sum_k q_k * isq_k^2  (per sample)
        nc.vector.tensor_tensor(out=isq2[:], in0=isq[:], in1=isq[:], op=ALU.mult)
        nc.vector.tensor_tensor(out=u[:], in0=q[:], in1=isq2[:], op=ALU.mult)
        g_p = pp_tiny.tile([NS, 1], f32, tag="tiny")
        nc.tensor.matmul(
            g_p[:nn, :], lhsT=u[:], rhs=ones_col[:], start=True, stop=True
        )
        # gis = g^-0.5 -> transpose to a row -> broadcast over k partitions
        gm = tails.tile([nn, 1], f32, tag=f"t_gm{n0}")
        nc.vector.tensor_scalar_max(gm[:], g_p[:nn, :], 1e-24)
        gis = tails.tile([nn, 1], f32, tag=f"t_gis{n0}")
        lgm = tails.tile([nn, 1], f32, tag=f"t_lgm{n0}")
        nc.scalar.activation(lgm[:], gm[:], AF.Ln)
        nc.scalar.activation(gis[:], lgm[:], AF.Exp, scale=-0.5)
        gr_p = pp_tiny.tile([1, NS], f32, tag="tiny")
        nc.tensor.matmul(
            gr_p[:, :nn], lhsT=gis[:], rhs=ident[:nn, :nn], start=True, stop=True
        )
        nc.vector.tensor_copy(gisr[:], gr_p[:, :nn])
        gb_p = pp_tiny.tile([K, NS], f32, tag="tiny")
        nc.tensor.matmul(
            gb_p[:, :nn], lhsT=ones_row[:], rhs=gisr[:], start=True, stop=True
        )
        nc.vector.tensor_copy(gb[:], gb_p[:, :nn])
        # s = isq * gb; vf = vl * s
        nc.vector.tensor_tensor(out=sall[:], in0=isq[:], in1=gb[:], op=ALU.mult)
        nc.gpsimd.tensor_tensor(out=vf[:], in0=vl[:], in1=bcast(sall[:], D), op=ALU.mult)
        nc.sync.dma_start(
            out=out_ap.rearrange("n k d -> k n d")[:, n0:n1, :], in_=vf[:]
        )

    # emission order per round: pass A of sample n FIRST (so its PSUM-evac
    # copies sit ahead of batch reduces in the DVE/ACT queues), then the
    # scalar chain of n-1, then the acc matmuls of n-3.
    PIPE = 3
    for n in range(NS):
        emit_load_and_passA(n)
        if n >= 1:
            emit_scalars(n - 1)
        if n >= PIPE:
            emit_passC(n - PIPE, agg_all, ssa_all)
            if n - PIPE == NS // 2 - 1:
                emit_tail(0, NS // 2)
    emit_passC(NS - PIPE, agg_all, ssa_all)
    emit_scalars(NS - 1)
    for n in range(NS - PIPE + 1, NS):
        emit_passC(n, agg_all, ssa_all)
    emit_tail(NS // 2, NS)


def kernel(x, conv_w, centroids):
    from concourse.bass_utils import run_bass_kernel_spmd

    if "nc" not in _CACHE:
        _CACHE["nc"] = _build_nc()
    nc = _CACHE["nc"]

    x = np.ascontiguousarray(np.asarray(x, dtype=np.float32).reshape(N, D, HW))
    wt = np.ascontiguousarray(np.asarray(conv_w, dtype=np.float32).T)
    cent = np.ascontiguousarray(np.asarray(centroids, dtype=np.float32))
    in_maps = [
        {"x": x[i * NS : (i + 1) * NS], "wt": wt, "cent": cent} for i in range(NCORES)
    ]
    res = run_bass_kernel_spmd(nc, in_maps, core_ids=list(range(NCORES))).results
    out = np.concatenate([r["out"].reshape(NS, K * D) for r in res], axis=0)
    return out


if __name__ == "__main__":
    rng = np.random.default_rng(0)
    xs = rng.standard_normal((N, D, 60, 80), dtype=np.float32)
    cw = (rng.standard_normal((K, D)) * 0.1).astype(np.float32)
    ct = rng.random((K, D), dtype=np.float32)
    o = kernel(x=xs, conv_w=cw, centroids=ct)
    print("kernel out", o.shape, o.dtype, np.abs(o).max())


# revision 43
# speedup vs baseline: 1.4746x; 1.4746x over previous
"""NetVLAD Trainium2 Bass kernel.

Full inputs in, full output out. Data-parallel over batch N=64 across 8
NeuronCores (8 samples per core); conv weight and centroids replicated.

Per-sample algorithm (mathematically equal to the reference, never
materializing the channel-normalized x):
  X = x[n]  [D=128, P=4800]  (D on SBUF partitions, contiguous in HBM)
  For each 128-wide chunk of P (p on partitions after a PE transpose):
    ss[p]    = sum_d X[d,p]^2
    inv_s    = ss^-0.5                   (DVE pow — keeps the ACT table
                                          set fixed: only Copy/Square/Exp)
    logitsT  = X_c^T @ Wt                (PE)
    e        = exp(logitsT * inv_s)      (softmax max-subtraction skipped:
                                          |logits*inv_s| <= ~1.2)
    sb       = e * (inv_s / Z),  Z = sum_k e
    acc     += [sb | e]^T @ [X_c^T | 1/Z]   (PE, PSUM accumulate)
  agg      = acc[0:64, 0:128];  sum_sa = acc[64:128, 128]
  vlad     = agg - sum_sa * centroids, then intra + global L2 norm.

Pipelining: per-chunk scalar work is batched into whole-sample ops
(one Square, one reduce, one Exp, ...); the accumulate matmuls of
sample n-2 are emitted between pass A of sample n so the PE never
waits on the scalar chain. The [sb|e] and [XT|1/Z] operands are bf16
(FWL fast weight load; f32 PSUM accumulation).
"""

import sys

if "/opt/trn_rl_repo" not in sys.path:
    sys.path.insert(0, "/opt/trn_rl_repo")

import numpy as np
from contextlib import ExitStack

N, D, HW, K = 64, 128, 4800, 64
NCORES = 8
NS = N // NCORES  # samples per core

CHUNKS = [(i * 128, min(128, HW - i * 128)) for i in range((HW + 127) // 128)]
NCH = len(CHUNKS)  # 38: 37 full + one 64-wide

_CACHE = {}


def _patch_act_tables():
    """Steer bacc's ACT table-set placement to the one set that covers
    every function we use (ln/exp/square/copy) so the kernel pays a single
    ACT_TABLE_LOAD instead of thrashing between per-anchor sets."""
    if _CACHE.get("act_patched"):
        return
    from concourse import bacc, mybir

    orig = bacc.get_activation_tables
    AF = mybir.ActivationFunctionType
    combo = "natural_log_exp_and_others"

    def patched(arch):
        t = {k: set(v) for k, v in orig(arch).items()}
        if combo in t:
            for name in t:
                if name != combo:
                    t[name] = t[name] - {AF.Ln, AF.Exp}
        return t

    bacc.get_activation_tables = patched
    _CACHE["act_patched"] = True


def _build_nc():
    import concourse.tile as tile
    from concourse import bacc, mybir

    _patch_act_tables()

    nc = bacc.Bacc(
        "TRN2",
        target_bir_lowering=False,
        debug=False,
        enable_asserts=False,
        num_devices=NCORES,
    )
    x_ap = nc.dram_tensor("x", [NS, D, HW], mybir.dt.float32, kind="ExternalInput").ap()
    wt_ap = nc.dram_tensor("wt", [D, K], mybir.dt.float32, kind="ExternalInput").ap()
    cent_ap = nc.dram_tensor(
        "cent", [K, D], mybir.dt.float32, kind="ExternalInput"
    ).ap()
    out_ap = nc.dram_tensor(
        "out", [NS, K, D], mybir.dt.float32, kind="ExternalOutput"
    ).ap()

    with tile.TileContext(nc) as tc:
        with ExitStack() as ctx:
            _body(ctx, tc, out_ap, x_ap, wt_ap, cent_ap)
    nc.compile()
    return nc


def _body(ctx, tc, out_ap, x_ap, wt_ap, cent_ap):
    import concourse.bass as bass
    from concourse import masks, mybir

    nc = tc.nc
    f32 = mybir.dt.float32
    bf16 = mybir.dt.bfloat16
    AF = mybir.ActivationFunctionType
    ALU = mybir.AluOpType
    X_AX = mybir.AxisListType.X

    singles = ctx.enter_context(tc.tile_pool(name="singles", bufs=1))
    xpool = ctx.enter_context(tc.tile_pool(name="xpool", bufs=2))
    xtrpool = ctx.enter_context(tc.tile_pool(name="xtrpool", bufs=4))
    ebpool = ctx.enter_context(tc.tile_pool(name="ebpool", bufs=2))
    sbtpool = ctx.enter_context(tc.tile_pool(name="sbtpool", bufs=4))
    lpool = ctx.enter_context(tc.tile_pool(name="lpool", bufs=2))
    scrpool = ctx.enter_context(tc.tile_pool(name="scrpool", bufs=2))
    smalls = ctx.enter_context(tc.tile_pool(name="smalls", bufs=3))
    tails = ctx.enter_context(tc.tile_pool(name="tails", bufs=1))
    pp_xt = ctx.enter_context(tc.tile_pool(name="pp_xt", bufs=6, space="PSUM"))
    pp_acc = ctx.enter_context(tc.tile_pool(name="pp_acc", bufs=1, space="PSUM"))
    pp_tiny = ctx.enter_context(tc.tile_pool(name="pp_tiny", bufs=1, space="PSUM"))

    def bcast(ap, n):
        # append a step-0 free dim: [..., n] broadcast view
        return bass.AP(tensor=ap.tensor, offset=ap.offset, ap=list(ap.ap) + [[0, n]])

    def mid_bcast(ap, n):
        # [p, f] -> [p, n, f] with step-0 middle dim
        return bass.AP(
            tensor=ap.tensor,
            offset=ap.offset,
            ap=[ap.ap[0], [0, n]] + list(ap.ap[1:]),
        )

    # constants
    ident = singles.tile([128, 128], f32)
    masks.make_identity(nc, ident[:])
    # fused rhs for pass A: [identity | Wt] — one matmul yields [X_c^T | logits]
    identwt = singles.tile([128, 192], f32)
    masks.make_identity(nc, identwt[:, 0:128])
    nc.sync.dma_start(out=identwt[:, 128:192], in_=wt_ap[:])
    cent_s = singles.tile([K, D], f32)
    nc.sync.dma_start(out=cent_s[:], in_=cent_ap[:])
    ones_col = singles.tile([K, 1], f32)
    nc.vector.memset(ones_col[:], 1.0)
    ones_row = singles.tile([1, K], f32)
    nc.vector.memset(ones_row[:], 1.0)

    GRP = 2  # fused-matmul chunks per PSUM bank (finer -> more PE runway)
    groups = []
    c0 = 0
    while c0 < NCH:
        groups.append(list(range(c0, min(c0 + GRP, NCH))))
        c0 += GRP

    state = {}  # per-sample live tiles

    def emit_load_and_passA(n):
        xs = xpool.tile([D, HW], f32, tag="xs")
        nc.sync.dma_start(out=xs[:, 0 : HW // 2], in_=x_ap[n, :, 0 : HW // 2])
        nc.sync.dma_start(out=xs[:, HW // 2 :], in_=x_ap[n, :, HW // 2 :])

        # [XT | s] per chunk (bf16): cols 0:128 = X_c^T, col 128 = ||x_p||
        xtr = xtrpool.tile([128, NCH, 129], bf16, tag="xtr")
        # softmax numerators e (bf16, contiguous for the 2x reduce)
        et = ebpool.tile([128, NCH, K], bf16, tag="et")
        # sb = e * inv_s/Z — the acc matmul's stationary operand
        sbt = sbtpool.tile([128, NCH, K], bf16, tag="sbt")
        # raw logits stash (bf16)
        lgs = lpool.tile([128, NCH, K], bf16, tag="lgs")
        # XT^2 scratch (bf16 — ss reduce gets the 2x DVE mode)
        x2t = scrpool.tile([128, NCH * 128], bf16, tag="x2t")
        # scaled-logits scratch (bf16)
        slgt = scrpool.tile([128, NCH * K], bf16, tag="slgt")

        for gi, grp in enumerate(groups):
            gn = len(grp)
            # one fused matmul per chunk: out cols 0:128 = X_c^T, 128:192 =
            # logits. 256-col stride keeps each 192-col output in one bank.
            xt_p = pp_xt.tile([128, GRP, 256], f32, tag="xt")
            for j, c in enumerate(grp):
                p0, w = CHUNKS[c]
                x_c = xs[:, p0 : p0 + w]
                nc.tensor.matmul(
                    xt_p[:w, j, 0:192],
                    lhsT=x_c,
                    rhs=identwt[:],
                    start=True,
                    stop=True,
                )
            gc = grp[0]
            # alternate evacuation between DVE and ACT so neither engine's
            # batch work starves the PE's PSUM recycling
            if gi % 2 == 0:
                nc.vector.tensor_copy(
                    xtr[:, gc : gc + gn, 0:128], xt_p[:, 0:gn, 0:128]
                )
                nc.scalar.copy(lgs[:, gc : gc + gn, :], xt_p[:, 0:gn, 128:192])
            else:
                nc.scalar.copy(xtr[:, gc : gc + gn, 0:128], xt_p[:, 0:gn, 0:128])
                nc.vector.tensor_copy(lgs[:, gc : gc + gn, :], xt_p[:, 0:gn, 128:192])

        state[n] = (xs, xtr, et, sbt, lgs, x2t, slgt)

    def emit_scalars(n):
        xs, xtr, et, sbt, lgs, x2t, slgt = state[n]
        ss = smalls.tile([128, NCH], f32, tag="ss")
        zz = smalls.tile([128, NCH], f32, tag="zz")
        is_ = smalls.tile([128, NCH], f32, tag="is")
        tsc = smalls.tile([128, NCH], bf16, tag="tsc")

        # everything split per half-sample: shorter dependency links, and
        # the static per-engine schedule interleaves across samples better
        halves = [(0, NCH // 2), (NCH // 2, NCH)]
        lns = smalls.tile([128, NCH], f32, tag="lns")
        x2vf = x2t[:].rearrange("p (c d) -> p c d", c=NCH)
        for h0, h1 in halves:
            nc.scalar.activation(x2vf[:, h0:h1, :], xtr[:, h0:h1, 0:128], AF.Square)
            nc.vector.tensor_reduce(
                out=ss[:, h0:h1], in_=x2vf[:, h0:h1, :], axis=X_AX, op=ALU.add
            )
            # inv_s = exp(-0.5*ln(ss)); Ln+Exp live in one ACT table set
            nc.scalar.activation(lns[:, h0:h1], ss[:, h0:h1], AF.Ln)
            nc.scalar.activation(is_[:, h0:h1], lns[:, h0:h1], AF.Exp, scale=-0.5)
            # s = ss * inv_s = ||x_p||, into col 128 of each xtr chunk (the
            # acc matmul's rhs column turning sb into sum_sa)
            nc.gpsimd.tensor_tensor(
                out=xtr[:, h0:h1, 128],
                in0=ss[:, h0:h1],
                in1=is_[:, h0:h1],
                op=ALU.mult,
            )

        for h0, h1 in halves:
            slg = slgt[:, h0 * K : h1 * K].rearrange("p (c k) -> p c k", c=h1 - h0)
            nc.gpsimd.tensor_tensor(
                out=slg,
                in0=lgs[:, h0:h1, :],
                in1=bcast(is_[:, h0:h1], K),
                op=ALU.mult,
            )
            nc.scalar.activation(et[:, h0:h1, :], slg, AF.Exp)
            nc.vector.tensor_reduce(
                out=zz[:, h0:h1], in_=et[:, h0:h1, :], axis=X_AX, op=ALU.add
            )
            rr = smalls.tile([128, NCH // 2], f32, tag="rr")
            nc.vector.reciprocal(rr[:], zz[:, h0:h1])
            # t = inv_s / Z
            nc.gpsimd.tensor_tensor(
                out=tsc[:, h0:h1], in0=is_[:, h0:h1], in1=rr[:], op=ALU.mult
            )
            # sb = e * t
            nc.gpsimd.tensor_tensor(
                out=sbt[:, h0:h1, :],
                in0=et[:, h0:h1, :],
                in1=bcast(tsc[:, h0:h1], K),
                op=ALU.mult,
            )

    def emit_passC(n, agg_all, ssa_all):
        xs, xtr, et, sbt, lgs, x2t, slgt = state.pop(n)
        acc_p = pp_acc.tile([K, 129], f32, tag="acc")
        for c, (p0, w) in enumerate(CHUNKS):
            nc.tensor.matmul(
                acc_p[:, :],
                lhsT=sbt[:w, c, :],
                rhs=xtr[:w, c, :],
                start=(c == 0),
                stop=(c == NCH - 1),
            )
        # evacuate: agg = cols 0:128; sum_sa = col 128
        nc.vector.tensor_copy(agg_all[:, n, :], acc_p[:, 0:D])
        nc.scalar.copy(ssa_all[:, n : n + 1], acc_p[:, 128:129])

    # batched across all samples
    agg_all = tails.tile([K, NS, D], f32)
    ssa_all = tails.tile([K, NS], f32)

    def emit_tail(n0, n1):
        nn = n1 - n0
        agg_h = agg_all[:, n0:n1, :]
        ssa_h = ssa_all[:, n0:n1]
        vl = tails.tile([K, nn, D], f32, tag=f"t_vl{n0}")
        vsq = tails.tile([K, nn * D], f32, tag=f"t_vsq{n0}")
        q = tails.tile([K, nn], f32, tag=f"t_q{n0}")
        qm = tails.tile([K, nn], f32, tag=f"t_qm{n0}")
        isq = tails.tile([K, nn], f32, tag=f"t_isq{n0}")
        isq2 = tails.tile([K, nn], f32, tag=f"t_isq2{n0}")
        u = tails.tile([K, nn], f32, tag=f"t_u{n0}")
        gisr = tails.tile([1, nn], f32, tag=f"t_gisr{n0}")
        gb = tails.tile([K, nn], f32, tag=f"t_gb{n0}")
        sall = tails.tile([K, nn], f32, tag=f"t_s{n0}")
        vf = tails.tile([K, nn, D], f32, tag=f"t_vf{n0}")

        # vl = agg - ssa * cent
        nc.gpsimd.tensor_tensor(
            out=vl[:], in0=bcast(ssa_h, D), in1=mid_bcast(cent_s[:], nn), op=ALU.mult
        )
        nc.vector.tensor_tensor(out=vl[:], in0=agg_h, in1=vl[:], op=ALU.subtract)
        # q = rowsum(vl^2) per (k, n)
        vsqv = vsq[:].rearrange("k (n d) -> k n d", n=nn)
        nc.scalar.activation(vsqv, vl[:], AF.Square)
        nc.vector.tensor_reduce(out=q[:], in_=vsqv, axis=X_AX, op=ALU.add)
        nc.vector.tensor_scalar_max(qm[:], q[:], 1e-24)
        lq = tails.tile([K, nn], f32, tag=f"t_lq{n0}")
        nc.scalar.activation(lq[:], qm[:], AF.Ln)
        nc.scalar.activation(isq[:], lq[:], AF.Exp, scale=-0.5)
        # g = sum_k q_k * isq_k^2  (per sample)
        nc.vector.tensor_tensor(out=isq2[:], in0=isq[:], in1=isq[:], op=ALU.mult)
        nc.vector.tensor_tensor(out=u[:], in0=q[:], in1=isq2[:], op=ALU.mult)
        g_p = pp_tiny.tile([NS, 1], f32, tag="tiny")
        nc.tensor.matmul(
            g_p[:nn, :], lhsT=u[:], rhs=ones_col[:], start=True, stop=True
        )
        # gis = g^-0.5 -> transpose to a row -> broadcast over k partitions
        gm = tails.tile([nn, 1], f32, tag=f"t_gm{n0}")
        nc.vector.tensor_scalar_max(gm[:], g_p[:nn, :], 1e-24)
        gis = tails.tile([nn, 1], f32, tag=f"t_gis{n0}")
        lgm = tails.tile([nn, 1], f32, tag=f"t_lgm{n0}")
        nc.scalar.activation(lgm[:], gm[:], AF.Ln)
        nc.scalar.activation(gis[:], lgm[:], AF.Exp, scale=-0.5)
        gr_p = pp_tiny.tile([1, NS], f32, tag="tiny")
        nc.tensor.matmul(
            gr_p[:, :nn], lhsT=gis[:], rhs=ident[:nn, :nn], start=True, stop=True
        )
        nc.vector.tensor_copy(gisr[:], gr_p[:, :nn])
        gb_p = pp_tiny.tile([K, NS], f32, tag="tiny")
        nc.tensor.matmul(
            gb_p[:, :nn], lhsT=ones_row[:], rhs=gisr[:], start=True, stop=True
        )
        nc.vector.tensor_copy(gb[:], gb_p[:, :nn])
        # s = isq * gb; vf = vl * s
        nc.vector.tensor_tensor(out=sall[:], in0=isq[:], in1=gb[:], op=ALU.mult)
        nc.gpsimd.tensor_tensor(out=vf[:], in0=vl[:], in1=bcast(sall[:], D), op=ALU.mult)
        nc.sync.dma_start(
            out=out_ap.rearrange("n k d -> k n d")[:, n0:n1, :], in_=vf[:]
        )

    # emission order per round: pass A of sample n FIRST (so its PSUM-evac
    # copies sit ahead of batch reduces in the DVE/ACT queues), then the
    # scalar chain of n-1, then the acc matmuls of n-3.
    PIPE = 3
    for n in range(NS):
        emit_load_and_passA(n)
        if n >= 1:
            emit_scalars(n - 1)
        if n >= PIPE:
            emit_passC(n - PIPE, agg_all, ssa_all)
            if n - PIPE == NS // 2 - 1:
                emit_tail(0, NS // 2)
    emit_passC(NS - PIPE, agg_all, ssa_all)
    emit_scalars(NS - 1)
    for n in range(NS - PIPE + 1, NS):
        emit_passC(n, agg_all, ssa_all)
    emit_tail(NS // 2, NS)


def kernel(x, conv_w, centroids):
    from concourse.bass_utils import run_bass_kernel_spmd

    if "nc" not in _CACHE:
        _CACHE["nc"] = _build_nc()
    nc = _CACHE["nc"]

    x = np.ascontiguousarray(np.asarray(x, dtype=np.float32).reshape(N, D, HW))
    wt = np.ascontiguousarray(np.asarray(conv_w, dtype=np.float32).T)
    cent = np.ascontiguousarray(np.asarray(centroids, dtype=np.float32))
    in_maps = [
        {"x": x[i * NS : (i + 1) * NS], "wt": wt, "cent": cent} for i in range(NCORES)
    ]
    res = run_bass_kernel_spmd(nc, in_maps, core_ids=list(range(NCORES))).results
    out = np.concatenate([r["out"].reshape(NS, K * D) for r in res], axis=0)
    return out


if __name__ == "__main__":
    rng = np.random.default_rng(0)
    xs = rng.standard_normal((N, D, 60, 80), dtype=np.float32)
    cw = (rng.standard_normal((K, D)) * 0.1).astype(np.float32)
    ct = rng.random((K, D), dtype=np.float32)
    o = kernel(x=xs, conv_w=cw, centroids=ct)
    print("kernel out", o.shape, o.dtype, np.abs(o).max())
